# revision 3
# baseline (speedup 1.0000x reference)
"""NashLoss2D on 8 TRN2 NeuronCores.

Inputs pred/targ are [10000, 5000] f32; targ has NaNs (missing obs).
Per station (column) j the loss needs four masked row-reductions:
    nansum_j = sum(isnan(targ))          -> cnt = NT - nansum
    s1_j     = sum(targ | nan->0)
    s2_j     = sum((targ | nan->0)^2)
    res_j    = sum(((targ - pred) | nan->0)^2)
then scalar finalization (mean/sst/valid/per_col) which is O(NS) and done
on the host in float64 (this also makes the reference's exact `sst != 0`
constant-column test robust).

Sharding: stations split 8 ways -> each core streams its [10000, 625] slab.
On-core layout: [time=125 partitions, 4*625 stations free]; the 4 stat
planes are reduced over partitions with ones-vector float32r matmuls
(full rate at N>=256) accumulating into PSUM across all 80 row-chunks.

Engine plan per block: DMA loads tg/pr; GpSimd computes the NaN mask;
DVE zeroes NaN lanes in place (copy_predicated) and the diff; ACT
produces the three value planes (copy/square) so every matmul waits on
exactly one engine semaphore (PE LDW tolerates very few sync waits).
"""

import sys
from contextlib import ExitStack

import numpy as np

sys.path.insert(0, "/opt/trn_rl_repo")

import concourse.bass as bass  # noqa: E402
import concourse.tile as tile  # noqa: E402
from concourse import bacc, mybir  # noqa: E402
from concourse.bass_utils import run_bass_kernel_spmd  # noqa: E402

NT = 10000  # timesteps (rows)
NS = 5000  # stations (cols)
NCORES = 8
SC = NS // NCORES  # 625 stations per core
SCP = 626  # padded station width (zero pad col): fp32r matmul needs even N
P = 125  # rows per chunk (SBUF partition dim); 10000 = 80 * 125 exactly
CH = 4  # row-chunks per block
BLK = P * CH  # 500 rows per block
NB = NT // BLK  # 20 blocks
W = CH * SCP  # free width of a block tile (2504)
# station pieces per chunk: both even (fp32r) and >=256 (fp32r full rate);
# third field is the free offset inside the PSUM tile (bank-aligned).
PIECES = ((0, 370, 0), (370, 626, 512))

_NC_CACHE = {}


def _build_nc():
    nc = bass.Bass()
    f32 = mybir.dt.float32
    f32r = mybir.dt.float32r
    i32 = mybir.dt.int32
    Act = mybir.ActivationFunctionType
    Op = mybir.AluOpType

    targ = nc.declare_dram_parameter("targ", [NT, SCP], f32, isOutput=False)
    pred = nc.declare_dram_parameter("pred", [NT, SCP], f32, isOutput=False)
    onesd = nc.declare_dram_parameter("ones", [P, 1], f32, isOutput=False)
    out = nc.declare_dram_parameter("out", [1, 4096], f32, isOutput=True)

    with ExitStack() as ctx:
        tc = ctx.enter_context(tile.TileContext(nc))
        singles = ctx.enter_context(tc.tile_pool(name="singles", bufs=1))
        work = ctx.enter_context(tc.tile_pool(name="work", bufs=2))
        psum = ctx.enter_context(tc.tile_pool(name="psum", bufs=1, space="PSUM"))

        # memset can't write f32r (invalid ISA), and fp32r matmul weights must
        # be produced "as f32r" — a DMA producer satisfies the verifier, so
        # ones comes from DRAM.
        ones = singles.tile([P, 1], f32r)
        nc.sync.dma_start(out=ones, in_=onesd[:].bitcast(f32r))
        zeros = singles.tile([P, W], f32)
        nc.vector.memset(zeros, 0.0)
        # stat j (0=nansum 1=s1 2=s2 3=res) piece p -> free [j*1024 + p*512 ..],
        # all on partition 0 (PE psum writes must start at partition 0/32/64).
        stats = psum.tile([1, 4096], f32)
        # SBUF bounce for the output; memset once so the gap regions are
        # initialized, pieces are overwritten from PSUM at the tail.
        fin = singles.tile([1, 4096], f32)
        nc.vector.memset(fin, 0.0)

        for b in range(NB):
            # only the four matmul planes are f32r-typed (walrus: CopyPredicated
            # rejects fp32r operands; fp32r matmul operands must be produced
            # as fp32r — TT/ACT outputs and DMA qualify)
            tg = work.tile([P, W], f32, tag="tg")
            pr = work.tile([P, W], f32, tag="pr")
            e = work.tile([P, W], f32, tag="e")
            mn = work.tile([P, W], f32r, tag="mn")
            tzc = work.tile([P, W], f32r, tag="tzc")
            t2 = work.tile([P, W], f32r, tag="t2")
            d2 = work.tile([P, W], f32r, tag="d2")

            r0 = b * BLK
            tgv = targ[r0 : r0 + BLK, :].rearrange("(c p) s -> p c s", p=P)
            prv = pred[r0 : r0 + BLK, :].rearrange("(c p) s -> p c s", p=P)
            nc.sync.dma_start(out=tg, in_=tgv)
            nc.sync.dma_start(out=pr, in_=prv)

            # mn = 1.0 where targ is NaN (NaN != NaN), else 0.0.
            # Everything elementwise lives on DVE/ACT only: each engine's
            # instruction stream then observes DMA ticks once, keeping every
            # instruction's emitted wait count within walrus' tiny budget.
            nc.vector.tensor_tensor(mn, tg, tg, Op.not_equal)
            # zero the NaN lanes of targ in place (mask viewed as int32 for
            # walrus; 1.0f/0.0f bits are nonzero/zero as int32)
            mni = mn[:].bitcast(i32)
            nc.vector.copy_predicated(tg, mni, zeros)
            # e = tz - pred (wrong at NaN lanes: 0 - pred), then zero those
            nc.vector.tensor_tensor(e, tg, pr, Op.subtract)
            nc.vector.copy_predicated(e, mni, zeros)
            # value planes finalized on ACT => single-producer for matmuls
            nc.scalar.copy(out=tzc, in_=tg)
            nc.scalar.activation(t2, tg, Act.Square)
            nc.scalar.activation(d2, e, Act.Square)

            planes = (mn, tzc, t2, d2)
            for c in range(CH):
                for j, pl in enumerate(planes):
                    for c0, c1, po in PIECES:
                        nc.tensor.matmul(
                            out=stats[0:1, j * 1024 + po : j * 1024 + po + (c1 - c0)],
                            lhsT=ones[:],
                            rhs=pl[:, c * SCP + c0 : c * SCP + c1],
                            start=(b == 0 and c == 0),
                            stop=(b == NB - 1 and c == CH - 1),
                        )

        # PSUM is not DMA-able: bounce written pieces through SBUF (all DVE so
        # the store DMA waits on a single semaphore)
        for j in range(4):
            for c0, c1, po in PIECES:
                o = j * 1024 + po
                nc.vector.tensor_copy(
                    out=fin[0:1, o : o + (c1 - c0)], in_=stats[0:1, o : o + (c1 - c0)]
                )
        nc.sync.dma_start(out=out[:], in_=fin)
    # Split excess on_wait entries onto InstEventSemaphore so every
    # instruction satisfies TRN2's wait-count limits (subset of Bacc.compile;
    # the full Bacc pipeline breaks fp32r self-loading matmuls).
    import bass_rust as _bass_rust

    _bass_rust.generate_event_semaphores(nc)
    return nc


def get_nc():
    if "nc" not in _NC_CACHE:
        _NC_CACHE["nc"] = _build_nc()
    return _NC_CACHE["nc"]


def _unpack(raw: np.ndarray) -> np.ndarray:
    """[1, 4096] device layout -> [4, SC] (stat j pieces at j*1024 + {0,512});
    drops the zero-pad station."""
    flat = raw.reshape(4096)
    rows = []
    for j in range(4):
        rows.append(
            np.concatenate(
                [flat[j * 1024 : j * 1024 + 370], flat[j * 1024 + 512 : j * 1024 + 768]]
            )[:SC]
        )
    return np.stack(rows)


def _finalize(stats: np.ndarray) -> np.ndarray:
    """stats: [4, NS] f32 device partials -> scalar f32 loss (host, f64)."""
    nansum, s1, s2, res = stats.astype(np.float64)
    cnt = NT - nansum
    cntf = np.maximum(cnt, 1.0)
    mean = s1 / cntf
    sst = s2 - s1 * mean
    valid = (cnt > 10) & (sst != 0.0)
    sst_safe = np.where(valid, np.maximum(sst, 0.0), 1.0)
    per_col = np.where(valid, res / (np.sqrt(sst_safe) + 0.1) ** 2, 0.0)
    n = valid.sum()
    return np.array(per_col.sum() / n, dtype=np.float32)


def make_in_maps(pred: np.ndarray, targ: np.ndarray) -> list:
    ones = np.ones((P, 1), dtype=np.float32)
    in_maps = []
    for c in range(NCORES):
        sl = slice(c * SC, (c + 1) * SC)
        pp = np.zeros((NT, SCP), dtype=np.float32)
        tp = np.zeros((NT, SCP), dtype=np.float32)
        pp[:, :SC] = pred[:, sl]
        tp[:, :SC] = targ[:, sl]
        in_maps.append({"pred": pp, "targ": tp, "ones": ones})
    return in_maps


def finalize_results(results: list) -> np.ndarray:
    stats = np.concatenate([_unpack(r["out"]) for r in results], axis=1)  # [4, NS]
    return _finalize(stats)


def kernel(pred: np.ndarray, targ: np.ndarray) -> np.ndarray:
    nc = get_nc()
    in_maps = make_in_maps(pred, targ)
    try:
        results = run_bass_kernel_spmd(nc, in_maps, list(range(NCORES))).results
    except Exception:
        # tile scheduling is not perfectly deterministic across processes; a
        # rebuild gives a fresh schedule if a rare bad one failed to compile
        _NC_CACHE.clear()
        nc = get_nc()
        results = run_bass_kernel_spmd(nc, in_maps, list(range(NCORES))).results
    return finalize_results(results)



# revision 7
# speedup vs baseline: 1.6984x; 1.6984x over previous
"""NashLoss2D on 8 TRN2 NeuronCores — transposed f16 streaming design.

Inputs pred/targ are [10000, 5000] f32; targ has NaNs (missing obs).
Per station (column) j the loss needs four masked row-reductions:
    cnt_j = sum(~isnan(targ))
    s1_j  = sum(tz)        tz = targ | nan->0
    s2_j  = sum(tz^2)
    res_j = sum(dz^2)      dz = tz - (pred | targ-nan-lanes->0)
then scalar finalization (mean/sst/valid/per_col), O(NS), host f64.

Design (vs the 459 us matmul-reduction baseline):
- Host pre-transposes each core's 625-station slab to station-major
  [625, 10000] and converts to f16 (gate is 2e-2 rel err; f16
  end-to-end sim error ~1e-7 since per-column errors average out over
  5000 columns). DMA bytes halve: 25 MB/core -> ~70 us at 360 GB/s.
- Stations on partitions (5 groups x 125), time on the free axis, so
  every reduction is a fused accum_out on the instruction that already
  does the elementwise work: no PE, no PSUM, no reduce passes.
- NaN handling without copy_predicated (probed on HW):
    * DVE ALU max/min are IEEE maxNum/minNum: max(NaN,0)=min(NaN,0)=0,
      so tz = max(tg,0) + min(tg,0) and the two tensor_scalar accums
      give s1 = s1a + s1b for free.
    * valid mask vm = (tg bitcast i16) <= 0x7C00 (NaNs from np's
      f32->f16 conversion are positive quiet NaNs, > 0x7C00 as int);
      its accum gives cnt for free.
  tensor_scalar runs in the DVE 4x perf mode on f16 (0.25 cyc/elem).
- Engine balance per [125 x 2500] tile pair (20 tiles/core):
    DVE : m1, m2, vm (ts 0.25 ea) + tz (tt 0.5) + dz on 15/20 tiles
    Pool: pz = pr * vm, + dz on 5/20 tiles
    ACT : Square(tz)+acc -> s2, Square(dz)+acc -> res
  -> DVE ~100 us, ACT ~100 us, Pool ~95 us, DMA ~70 us.
"""

import sys
from contextlib import ExitStack

import numpy as np

sys.path.insert(0, "/opt/trn_rl_repo")

import concourse.bass as bass  # noqa: E402
import concourse.tile as tile  # noqa: E402
from concourse import mybir  # noqa: E402
from concourse.bass_utils import run_bass_kernel_spmd  # noqa: E402

NT = 10000  # timesteps
NS = 5000  # stations
NCORES = 8
SC = NS // NCORES  # 625 stations per core
G = 5  # station groups per core (125 partitions each)
P = 125  # partitions (stations per group)
F = 2500  # time-chunk width (free axis)
C = NT // F  # 4 time chunks
NTILE = G * C  # 20 tiles per tensor per core

_NC_CACHE = {}


def _build_nc():
    nc = bass.Bass()
    f16 = mybir.dt.float16
    f32 = mybir.dt.float32
    i16 = mybir.dt.int16
    Act = mybir.ActivationFunctionType
    Op = mybir.AluOpType

    targ = nc.declare_dram_parameter("targ", [SC, NT], f16, isOutput=False)
    pred = nc.declare_dram_parameter("pred", [SC, NT], f16, isOutput=False)
    out = nc.declare_dram_parameter("out", [P, 100], f32, isOutput=True)

    with ExitStack() as ctx:
        tc = ctx.enter_context(tile.TileContext(nc))
        singles = ctx.enter_context(tc.tile_pool(name="singles", bufs=1))
        work = ctx.enter_context(tc.tile_pool(name="work", bufs=2))

        # accum_out slots, split by writing engine so the dep tracker never
        # orders instructions across engines through a shared tile.
        # dve_acc cols: cnt at slot, s1a at 20+slot, s1b at 40+slot
        # act_acc cols: s2 at slot, res at 20+slot
        dve_acc = singles.tile([P, 60], f32)
        act_acc = singles.tile([P, 40], f32)

        for g in range(G):
            for c in range(C):
                slot = g * C + c
                tg = work.tile([P, F], f16, tag="tg")
                pr = work.tile([P, F], f16, tag="pr")
                m1 = work.tile([P, F], f16, tag="m1")
                m2 = work.tile([P, F], f16, tag="m2")
                vm = work.tile([P, F], f16, tag="vm")
                tz = work.tile([P, F], f16, tag="tz")
                pz = work.tile([P, F], f16, tag="pz")
                dz = work.tile([P, F], f16, tag="dz")
                sqa = work.tile([P, F], f16, tag="sqa")
                sqb = work.tile([P, F], f16, tag="sqb")

                r0, t0 = g * P, c * F
                nc.sync.dma_start(out=tg, in_=targ[r0 : r0 + P, t0 : t0 + F])
                nc.sync.dma_start(out=pr, in_=pred[r0 : r0 + P, t0 : t0 + F])

                # m1/m2 halves (NaN -> 0 per IEEE maxNum); accums give s1
                nc.vector.tensor_scalar(
                    out=m1, in0=tg, scalar1=0.0, op0=Op.max, scalar2=None,
                    op1=Op.add, accum_out=dve_acc[:, 20 + slot : 21 + slot],
                )
                nc.vector.tensor_scalar(
                    out=m2, in0=tg, scalar1=0.0, op0=Op.min, scalar2=None,
                    op1=Op.add, accum_out=dve_acc[:, 40 + slot : 41 + slot],
                )
                # valid mask + cnt
                nc.vector.tensor_scalar(
                    out=vm, in0=tg[:].bitcast(i16), scalar1=31744.0, op0=Op.is_le,
                    scalar2=None, op1=Op.add, accum_out=dve_acc[:, slot : slot + 1],
                )
                nc.vector.tensor_tensor(tz, m1, m2, Op.add)
                # pz = pred masked to targ's valid lanes
                nc.gpsimd.tensor_tensor(pz, pr, vm, Op.mult)
                # dz = tz - pz; every 4th tile on Pool to balance engines
                if slot % 4 == 2:
                    nc.gpsimd.tensor_tensor(dz, tz, pz, Op.subtract)
                else:
                    nc.vector.tensor_tensor(dz, tz, pz, Op.subtract)
                nc.scalar.activation(
                    sqa, tz, Act.Square, accum_out=act_acc[:, slot : slot + 1]
                )
                nc.scalar.activation(
                    sqb, dz, Act.Square, accum_out=act_acc[:, 20 + slot : 21 + slot]
                )

        nc.sync.dma_start(out=out[:, 0:60], in_=dve_acc)
        nc.sync.dma_start(out=out[:, 60:100], in_=act_acc)

    import bass_rust as _bass_rust

    _bass_rust.generate_event_semaphores(nc)
    return nc


def get_nc():
    if "nc" not in _NC_CACHE:
        _NC_CACHE["nc"] = _build_nc()
    return _NC_CACHE["nc"]


def make_in_maps(pred: np.ndarray, targ: np.ndarray) -> list:
    in_maps = []
    for c in range(NCORES):
        sl = slice(c * SC, (c + 1) * SC)
        in_maps.append(
            {
                "pred": np.ascontiguousarray(pred[:, sl].T).astype(np.float16),
                "targ": np.ascontiguousarray(targ[:, sl].T).astype(np.float16),
            }
        )
    return in_maps


def _unpack(raw: np.ndarray) -> np.ndarray:
    """[125, 100] device accum slots -> [4, SC] stats (cnt, s1, s2, res).

    Column layout: [cnt(0:20) | s1a(20:40) | s1b(40:60) | s2(60:80) |
    res(80:100)], slot = g*C + c. Station index = g*125 + p.
    """

    def blk(j):
        b = raw[:, j * 20 : (j + 1) * 20].astype(np.float64)
        return b.reshape(P, G, C).sum(axis=2).T.reshape(SC)  # s = g*125 + p

    cnt = blk(0)
    s1 = blk(1) + blk(2)
    s2 = blk(3)
    res = blk(4)
    return np.stack([cnt, s1, s2, res])


def _finalize(stats: np.ndarray) -> np.ndarray:
    """stats: [4, NS] f64 (cnt, s1, s2, res) -> scalar f32 loss (host, f64)."""
    cnt, s1, s2, res = stats
    cntf = np.maximum(cnt, 1.0)
    mean = s1 / cntf
    sst = s2 - s1 * mean
    valid = (cnt > 10) & (sst != 0.0)
    sst_safe = np.where(valid, np.maximum(sst, 0.0), 1.0)
    per_col = np.where(valid, res / (np.sqrt(sst_safe) + 0.1) ** 2, 0.0)
    n = valid.sum()
    return np.array(per_col.sum() / n, dtype=np.float32)


def finalize_results(results: list) -> np.ndarray:
    stats = np.concatenate([_unpack(r["out"]) for r in results], axis=1)  # [4, NS]
    return _finalize(stats)


def kernel(pred: np.ndarray, targ: np.ndarray) -> np.ndarray:
    nc = get_nc()
    in_maps = make_in_maps(pred, targ)
    try:
        results = run_bass_kernel_spmd(nc, in_maps, list(range(NCORES))).results
    except Exception:
        # tile scheduling is not perfectly deterministic across processes; a
        # rebuild gives a fresh schedule if a rare bad one failed to compile
        _NC_CACHE.clear()
        nc = get_nc()
        results = run_bass_kernel_spmd(nc, in_maps, list(range(NCORES))).results
    return finalize_results(results)
